# revision 1
# baseline (speedup 1.0000x reference)
"""CombinedRotaryEmbedding Trainium2 kernel.

Math (per 64-dim head, per position s):
    xh  = x @ R                 (R = composed Givens rotations @ rotation_matrix)
    u   = xh[..., 0::2]  = x @ R[:, 0::2]
    v   = xh[..., 1::2]  = x @ R[:, 1::2]
    out = [u*cos - v*sin | u*sin + v*cos]      cos/sin = f(position, freq[32])

Kernel strategy (8-way data parallel over the sequence dim):
  - host: compose R (tiny [64,64]), build R2 = [R_even | R_odd]; build per-core
    cos/sin tables CC = [cos|cos], SS = [sin|-sin] laid out per SBUF partition.
  - device, per core (x shard [2048 rows, 1024]):
      PE   : transpose x [128,128] chunks (feat -> partitions)
      ACT  : copy transposed chunks PSUM -> SBUF
      PE   : per head, y2[128 rows, 64] = xT_head.T @ R2   ([u|v] layout)
      DVE  : t1 = y2*CC, t2 = y2*SS   (PSUM -> SBUF)
      GPSIMD: out_lo = t1_lo + t2_hi ; out_hi = t1_hi + t2_lo
      DMA  : contiguous row-tile loads/stores (nc.sync HWDGE)
"""

import numpy as np

import concourse.bacc as bacc
import concourse.bass as bass
import concourse.tile as tile
from concourse import mybir
from concourse.bass_utils import run_bass_kernel_spmd
from concourse.masks import make_identity

N_CORES = 8
B, S, N_STATE = 4, 4096, 1024
H, D = 16, 64           # heads, head dim
HALF = D // 2           # 32 rotary freqs
S_SH = S // N_CORES     # 512 positions per core
ROWS = B * S_SH         # 2048 rows of [1024] per core
RT = ROWS // 128        # 16 row tiles
CBLK = S_SH // 128      # 4 distinct position blocks per core
F32 = mybir.dt.float32

_compiled = {}


def _build_nc(adds_on="gpsimd", xin_bufs=4, outp_bufs=4, ysb_bufs=4,
              tmp_bufs=6, xtp_bufs=2, tpsum_bufs=2, ypsum_bufs=3,
              out_split=2, ablate="full"):
    nc = bacc.Bacc("TRN2")
    x_in = nc.dram_tensor("x", [ROWS, N_STATE], F32, kind="ExternalInput")
    # r2 arrives as blockdiag(R2, R2) so one K=128 matmul covers 2 heads
    r2_in = nc.dram_tensor("r2", [128, 128], F32, kind="ExternalInput")
    # ccss[p, c, 0] = [cos|cos], ccss[p, c, 1] = [sin|-sin] for position c*128+p
    ccss_in = nc.dram_tensor("ccss", [128, CBLK, 2, D], F32, kind="ExternalInput")
    out_d = nc.dram_tensor("out", [ROWS, N_STATE], F32, kind="ExternalOutput")

    with tile.TileContext(nc) as tc:
        with (
            tc.tile_pool(name="const", bufs=1) as const,
            tc.tile_pool(name="xin", bufs=xin_bufs) as xin,
            tc.tile_pool(name="xtp", bufs=xtp_bufs) as xtp,
            tc.tile_pool(name="tpsum", bufs=tpsum_bufs, space="PSUM") as tpsum,
            tc.tile_pool(name="ypsum", bufs=ypsum_bufs, space="PSUM") as ypsum,
            tc.tile_pool(name="ysb", bufs=ysb_bufs) as ysb,
            tc.tile_pool(name="tmp", bufs=tmp_bufs) as tmp,
            tc.tile_pool(name="outp", bufs=outp_bufs) as outp,
        ):
            ident = const.tile([128, 128], F32)
            make_identity(nc, ident[:])
            r2_sb = const.tile([128, 128], F32)
            nc.sync.dma_start(out=r2_sb[:], in_=r2_in[:])
            # ccss_sb[p, c*2+t, 0:128] = ccss_in[p, c, t, :] duplicated twice
            # (DMA re-reads the 64-wide row via a step-0 dim)
            ccss_sb = const.tile([128, CBLK * 2, 2, D], F32)
            ccss_src = ccss_in[:]
            nc.sync.dma_start(
                out=ccss_sb[:],
                in_=bass.AP(
                    tensor=ccss_src.tensor, offset=ccss_src.offset,
                    ap=[list(ccss_src.ap[0]), [D, CBLK * 2], [0, 2], [1, D]],
                ),
            )

            for rt in range(RT):
                c = rt % CBLK
                x_t = xin.tile([128, N_STATE], F32)
                nc.sync.dma_start(out=x_t[:], in_=x_in[rt * 128:(rt + 1) * 128, :])

                if ablate == "dma":
                    ot = outp.tile([128, N_STATE], F32, tag="otd")
                    nc.vector.tensor_copy(ot[:], x_t[:])
                    nc.sync.dma_start(
                        out=out_d[rt * 128:(rt + 1) * 128], in_=ot[:])
                    continue

                # transpose 8 x [128,128] chunks; feats end up on partitions
                xT = xtp.tile([128, 8, 128], F32)
                for g in range(2):
                    tp = tpsum.tile([128, 4, 128], F32)
                    for q in range(4):
                        ch = g * 4 + q
                        nc.tensor.transpose(
                            tp[:, q, :],
                            x_t[:, ch * 128:(ch + 1) * 128],
                            ident[:],
                        )
                    nc.scalar.copy(out=xT[:, g * 4:(g + 1) * 4, :], in_=tp[:])

                # out_t [p, g2, j, b, e]: head = g2*4 + 2j + b, flat = natural
                out_t = outp.tile([128, 4, 2, 2, D], F32)
                for g2 in range(4):
                    # one matmul per PSUM bank (HW: >1 matmul/bank is fatal);
                    # each matmul computes 2 heads via the block-diagonal rhs
                    y2p = ypsum.tile([128, 2, 512], F32)
                    for j in range(2):
                        nc.tensor.matmul(
                            y2p[:, j, 0:128], xT[:, g2 * 2 + j, :], r2_sb[:],
                            start=True, stop=True,
                        )
                    # one fused DVE op: t12[t] = y2 * (cos-row if t==0 else sin-row)
                    # in0 doubles the psum read via a step-0 leading dim
                    y2ap = y2p[:, :, 0:128]
                    t12 = tmp.tile([128, 2, 2, 128], F32, tag="t12")
                    nc.vector.tensor_mul(
                        t12[:],
                        bass.AP(tensor=y2ap.tensor, offset=y2ap.offset,
                                ap=[list(y2ap.ap[0]), [0, 2], [512, 2],
                                    [1, 128]]),
                        bass.AP(tensor=ccss_sb.tensor,
                                offset=ccss_sb[:].offset + c * 256,
                                ap=[list(ccss_sb[:].ap[0]), [128, 2], [0, 2],
                                    [1, 128]]),
                    )
                    # one fused gpsimd op: crossed add, in1 reads swapped
                    # 32-halves via a negative mid-stride
                    og = out_t[:, g2]
                    t12a = t12[:]
                    eng = nc.gpsimd if adds_on == "gpsimd" else nc.vector
                    eng.tensor_tensor(
                        out=bass.AP(tensor=og.tensor, offset=og.offset,
                                    ap=[list(og.ap[0]), [D, 4], [HALF, 2],
                                        [1, HALF]]),
                        in0=bass.AP(tensor=t12a.tensor, offset=t12a.offset,
                                    ap=[list(t12a.ap[0]), [D, 4], [HALF, 2],
                                        [1, HALF]]),
                        in1=bass.AP(tensor=t12a.tensor,
                                    offset=t12a.offset + 256 + HALF,
                                    ap=[list(t12a.ap[0]), [D, 4], [-HALF, 2],
                                        [1, HALF]]),
                        op=mybir.AluOpType.add,
                    )
                    if out_split > 1 and g2 % (4 // out_split) == (4 // out_split) - 1:
                        w = N_STATE // out_split
                        s = g2 // (4 // out_split)
                        flat = out_t[:].rearrange("p a b c d -> p (a b c d)")
                        nc.sync.dma_start(
                            out=out_d[rt * 128:(rt + 1) * 128,
                                      s * w:(s + 1) * w],
                            in_=flat[:, s * w:(s + 1) * w])
                flat = out_t[:].rearrange("p a b c d -> p (a b c d)")
                if out_split == 1:
                    nc.sync.dma_start(
                        out=out_d[rt * 128:(rt + 1) * 128], in_=flat)
    nc.compile()  # bacc: splits multi-sem waits into EventSemaphore insts
    return nc


def _compose_r2(thetas, rotation_pairs, theta_scale, rotation_matrix):
    """Replicates reference._compose_rotation, then permutes cols to [even|odd]."""
    idx = rotation_pairs.astype(np.int32)
    th = (thetas.astype(np.float32) * np.float32(theta_scale[0]))
    R = np.eye(D, dtype=np.float32)
    for k in range(th.shape[0]):
        i, j = int(idx[k, 0]), int(idx[k, 1])
        ck, sk = np.float32(np.cos(th[k])), np.float32(np.sin(th[k]))
        G = np.eye(D, dtype=np.float32)
        G[i, i] = ck
        G[i, j] = -sk
        G[j, i] = sk
        G[j, j] = ck
        R = (R @ G).astype(np.float32)
    R = (R @ rotation_matrix.astype(np.float32)).astype(np.float32)
    return np.ascontiguousarray(
        np.concatenate([R[:, 0::2], R[:, 1::2]], axis=1), dtype=np.float32
    )


def _pos_tables(inv_freq):
    pos = np.arange(S, dtype=np.float32)
    sinusoid = pos[:, None] * inv_freq[None, :].astype(np.float32)  # [S, 32]
    return np.cos(sinusoid).astype(np.float32), np.sin(sinusoid).astype(np.float32)


def _ccss_layout(cos_blk, sin_blk):
    """[512, 32] cos/sin tables -> [128 part, CBLK, 2, 64] compact layout:
    row t=0 is [cos|cos], row t=1 is [sin|-sin], partition = pos % 128."""
    cc = np.concatenate([cos_blk, cos_blk], axis=1)           # [512, 64]
    ss = np.concatenate([sin_blk, -sin_blk], axis=1)
    t = np.stack([cc, ss], axis=1)                            # [512, 2, 64]
    t = t.reshape(CBLK, 128, 2, D).transpose(1, 0, 2, 3)      # [128, CBLK, 2, 64]
    return np.ascontiguousarray(t, dtype=np.float32)


def make_in_maps(x, thetas, rotation_pairs, theta_scale, rotation_matrix,
                 inv_freq):
    x = np.asarray(x, dtype=np.float32)
    r2s = _compose_r2(
        np.asarray(thetas, np.float32),
        np.asarray(rotation_pairs, np.float32),
        np.asarray(theta_scale, np.float32),
        np.asarray(rotation_matrix, np.float32),
    )
    r2 = np.zeros((128, 128), dtype=np.float32)
    r2[0:D, 0:D] = r2s
    r2[D:128, D:128] = r2s
    cosf, sinf = _pos_tables(np.asarray(inv_freq, np.float32))

    in_maps = []
    for k in range(N_CORES):
        blk = slice(k * S_SH, (k + 1) * S_SH)
        ccss = _ccss_layout(cosf[blk], sinf[blk])
        xs = np.ascontiguousarray(x[:, blk, :]).reshape(ROWS, N_STATE)
        in_maps.append({"x": xs, "r2": r2, "ccss": ccss})
    return in_maps


def kernel(x, thetas, rotation_pairs, theta_scale, rotation_matrix, inv_freq):
    in_maps = make_in_maps(x, thetas, rotation_pairs, theta_scale,
                           rotation_matrix, inv_freq)
    if "nc" not in _compiled:
        _compiled["nc"] = _build_nc()
    res = run_bass_kernel_spmd(_compiled["nc"], in_maps, list(range(N_CORES))).results

    out = np.empty((B, S, N_STATE), dtype=np.float32)
    for k in range(N_CORES):
        blk = slice(k * S_SH, (k + 1) * S_SH)
        out[:, blk, :] = res[k]["out"].reshape(B, S_SH, N_STATE)
    return out



# revision 2
# speedup vs baseline: 1.4850x; 1.4850x over previous
"""CombinedRotaryEmbedding Trainium2 kernel (fp16 I/O, host-pretransposed).

Math (per 64-dim head, position s):
    y   = x @ R2            R2 = [R_even | R_odd]  ->  y = [u(32) | v(32)]
    out = [u*cos - v*sin | u*sin + v*cos]          cos/sin = f(position, freq)

Strategy (8-way data parallel over the sequence dim, 512 positions/core):
  - host: compose R (tiny [64,64]); build blockdiag([R2,R2]) in fp16; cast +
    pre-transpose x to fp16 with features on partitions so the device needs
    no transpose pass; build per-core compact cos/sin tables
    T[pos%128, blk, 0] = [cos|-sin], T[.,.,1] = [sin|cos] (64 wide, fp16).
  - device, per core (16 row tiles of 128 rows x 1024 feats):
      DMA : fp16 in/out, fully contiguous 4KB/partition transfers
      PE  : 8 fp16 matmuls [128x128x128] per row tile (y2 into PSUM fp32),
            one matmul per 2KB PSUM bank
      ACT : copy PSUM fp32 -> SBUF fp16 (2 strided copies per row tile)
      DVE : fused mul t12[t] = y * T[t] (one op, free=2048, fp16 2x mode)
      DVE/GPSIMD (alternating): crossed add out = t12_u + t12_v via
            strided APs (one op, free=1024)
  - Elementwise tables fold the [u|v] -> [lo|hi] pairing so a single add
    finishes the rotary: t12[0] = [u*c | -v*s], t12[1] = [u*s | v*c];
    out_lo = t12[0,u]+t12[0,v], out_hi = t12[1,u]+t12[1,v].
"""

import numpy as np

import concourse.bacc as bacc
import concourse.bass as bass
import concourse.tile as tile
from concourse import mybir
from concourse.bass_utils import run_bass_kernel_spmd

N_CORES = 8
B, S, N_STATE = 4, 4096, 1024
H, D = 16, 64           # heads, head dim
HALF = D // 2           # 32 rotary freqs
S_SH = S // N_CORES     # 512 positions per core
ROWS = B * S_SH         # 2048 rows of [1024] per core
RT = ROWS // 128        # 16 row tiles
DBL = RT // 2           # 8 double row tiles (DMA granularity)
CBLK = S_SH // 128      # 4 distinct position blocks per core
F32 = mybir.dt.float32
F16 = mybir.dt.float16

_compiled = {}


def _build_nc():
    nc = bacc.Bacc("TRN2")
    # x pre-transposed+tiled on host: [d, p=feat%128, j=rt%2, g=chunk, r=row]
    x_in = nc.dram_tensor("x", [DBL, 128, 2, 8, 128], F16, kind="ExternalInput")
    # blockdiag(R2, R2): one K=128 matmul covers 2 heads
    r2_in = nc.dram_tensor("r2", [128, 128], F16, kind="ExternalInput")
    # ccss[p, blk, 0] = [cos|-sin], ccss[p, blk, 1] = [sin|cos]  (pos=blk*128+p)
    ccss_in = nc.dram_tensor("ccss", [128, CBLK, 2, D], F16, kind="ExternalInput")
    # out[d, p=row%128, j, col]; host un-permutes
    out_d = nc.dram_tensor("out", [DBL, 128, 2, N_STATE], F16,
                           kind="ExternalOutput")

    with tile.TileContext(nc) as tc:
        with (
            tc.tile_pool(name="const", bufs=1) as const,
            tc.tile_pool(name="xin", bufs=3) as xin,
            tc.tile_pool(name="ypsum", bufs=2, space="PSUM") as ypsum,
            tc.tile_pool(name="yfp", bufs=3) as yfp,
            tc.tile_pool(name="t12p", bufs=3) as t12p,
            tc.tile_pool(name="outp", bufs=3) as outp,
        ):
            r2_sb = const.tile([128, 128], F16)
            nc.sync.dma_start(out=r2_sb[:], in_=r2_in[:])
            ccss_sb = const.tile([128, CBLK, 2, D], F16)
            nc.sync.dma_start(out=ccss_sb[:], in_=ccss_in[:])
            ccss_a = ccss_sb[:]

            for d in range(DBL):
                x_t = xin.tile([128, 2, 8, 128], F16, tag="x")
                nc.sync.dma_start(out=x_t[:], in_=x_in[d])
                out_t = outp.tile([128, 2, N_STATE], F16, tag="o")
                for j in range(2):
                    rt = d * 2 + j
                    c = rt % CBLK
                    yf = yfp.tile([128, N_STATE], F16, tag="yf")
                    for h in range(2):
                        yp = ypsum.tile([128, 4, 512], F32, tag="yp")
                        for q in range(4):
                            g = h * 4 + q
                            nc.tensor.matmul(
                                yp[:, q, 0:128], x_t[:, j, g, :], r2_sb[:],
                                start=True, stop=True,
                            )
                        # strided PSUM read packs the 4 banks' [128,128] blocks
                        ypa = yp[:]
                        yfa = yf[:]
                        nc.scalar.copy(
                            out=bass.AP(tensor=yfa.tensor,
                                        offset=yfa.offset + h * 512,
                                        ap=[list(yfa.ap[0]), [128, 4], [1, 128]]),
                            in_=bass.AP(tensor=ypa.tensor, offset=ypa.offset,
                                        ap=[list(ypa.ap[0]), [512, 4], [1, 128]]),
                        )
                    # one fused DVE mul: t12[t, hd, f] = y[hd, f] * T[t, f]
                    # (in0 re-reads y via a step-0 t dim; in1 re-reads the
                    #  64-wide table per head via a step-0 head dim)
                    t12 = t12p.tile([128, 2, N_STATE], F16, tag="t12")
                    yfa = yf[:]
                    nc.vector.tensor_mul(
                        t12[:],
                        bass.AP(tensor=yfa.tensor, offset=yfa.offset,
                                ap=[list(yfa.ap[0]), [0, 2], [D, H], [1, D]]),
                        bass.AP(tensor=ccss_a.tensor,
                                offset=ccss_a.offset + c * 2 * D,
                                ap=[list(ccss_a.ap[0]), [D, 2], [0, H], [1, D]]),
                    )
                    # one crossed add: out[t, hd, 0:32 or 32:64] =
                    #   t12[t, hd, u] + t12[t, hd, v]
                    t12a = t12[:]
                    og = out_t[:, j, :]
                    eng = nc.vector if j == 0 else nc.gpsimd
                    eng.tensor_tensor(
                        out=bass.AP(tensor=og.tensor, offset=og.offset,
                                    ap=[list(og.ap[0]), [HALF, 2], [D, H],
                                        [1, HALF]]),
                        in0=bass.AP(tensor=t12a.tensor, offset=t12a.offset,
                                    ap=[list(t12a.ap[0]), [N_STATE, 2], [D, H],
                                        [1, HALF]]),
                        in1=bass.AP(tensor=t12a.tensor,
                                    offset=t12a.offset + HALF,
                                    ap=[list(t12a.ap[0]), [N_STATE, 2], [D, H],
                                        [1, HALF]]),
                        op=mybir.AluOpType.add,
                    )
                nc.scalar.dma_start(out=out_d[d], in_=out_t[:])
    nc.compile()
    return nc


def _compose_r2(thetas, rotation_pairs, theta_scale, rotation_matrix):
    """Replicates reference._compose_rotation, then permutes cols to [even|odd]."""
    idx = rotation_pairs.astype(np.int32)
    th = (thetas.astype(np.float32) * np.float32(theta_scale[0]))
    R = np.eye(D, dtype=np.float32)
    for k in range(th.shape[0]):
        i, j = int(idx[k, 0]), int(idx[k, 1])
        ck, sk = np.float32(np.cos(th[k])), np.float32(np.sin(th[k]))
        G = np.eye(D, dtype=np.float32)
        G[i, i] = ck
        G[i, j] = -sk
        G[j, i] = sk
        G[j, j] = ck
        R = (R @ G).astype(np.float32)
    R = (R @ rotation_matrix.astype(np.float32)).astype(np.float32)
    return np.ascontiguousarray(
        np.concatenate([R[:, 0::2], R[:, 1::2]], axis=1), dtype=np.float32
    )


def make_in_maps(x, thetas, rotation_pairs, theta_scale, rotation_matrix,
                 inv_freq):
    x = np.asarray(x, dtype=np.float32)
    r2s = _compose_r2(
        np.asarray(thetas, np.float32),
        np.asarray(rotation_pairs, np.float32),
        np.asarray(theta_scale, np.float32),
        np.asarray(rotation_matrix, np.float32),
    )
    r2 = np.zeros((128, 128), dtype=np.float32)
    r2[0:D, 0:D] = r2s
    r2[D:128, D:128] = r2s
    r2 = r2.astype(np.float16)

    pos = np.arange(S, dtype=np.float32)
    sinusoid = pos[:, None] * np.asarray(inv_freq, np.float32)[None, :]  # [S,32]
    cosf = np.cos(sinusoid).astype(np.float32)
    sinf = np.sin(sinusoid).astype(np.float32)

    in_maps = []
    for k in range(N_CORES):
        blk = slice(k * S_SH, (k + 1) * S_SH)
        cb, sb = cosf[blk], sinf[blk]                       # [512, 32]
        t0 = np.concatenate([cb, -sb], axis=1)              # [512, 64]
        t1 = np.concatenate([sb, cb], axis=1)
        ccss = np.stack([t0, t1], axis=1)                   # [512, 2, 64]
        ccss = ccss.reshape(CBLK, 128, 2, D).transpose(1, 0, 2, 3)
        ccss = np.ascontiguousarray(ccss, dtype=np.float16)  # [128, 4, 2, 64]

        xs = x[:, blk, :].reshape(B, CBLK, 128, 8, 128)     # [b, sblk, r, g, p]
        xs = xs.transpose(0, 1, 4, 3, 2).reshape(DBL, 2, 128, 8, 128)
        xs = np.ascontiguousarray(
            xs.transpose(0, 2, 1, 3, 4), dtype=np.float16)  # [d, p, j, g, r]
        in_maps.append({"x": xs, "r2": r2, "ccss": ccss})
    return in_maps


def kernel(x, thetas, rotation_pairs, theta_scale, rotation_matrix, inv_freq):
    in_maps = make_in_maps(x, thetas, rotation_pairs, theta_scale,
                           rotation_matrix, inv_freq)
    if "nc" not in _compiled:
        _compiled["nc"] = _build_nc()
    res = run_bass_kernel_spmd(_compiled["nc"], in_maps, list(range(N_CORES))).results

    out = np.empty((B, S, N_STATE), dtype=np.float32)
    for k in range(N_CORES):
        blk = slice(k * S_SH, (k + 1) * S_SH)
        o = res[k]["out"]                                   # [d, p, j, col] f16
        o = o.transpose(0, 2, 1, 3).reshape(B, S_SH, N_STATE)
        out[:, blk, :] = o.astype(np.float32)
    return out


# revision 11
# speedup vs baseline: 2.0204x; 1.3606x over previous
"""CombinedRotaryEmbedding Trainium2 kernel (fp16 I/O, host-pretransposed).

Math (per 64-dim head, position s):
    y   = x @ R2            R2 = [R_even | R_odd]  ->  y = [u(32) | v(32)]
    out = [u*cos - v*sin | u*sin + v*cos]          cos/sin = f(position, freq)

Strategy (8-way data parallel over the sequence dim, 512 positions/core):
  - host: compose R (tiny [64,64]); build blockdiag([R2,R2]) in fp16; cast +
    pre-transpose x to fp16 with features on partitions so the device needs
    no transpose pass; build per-core compact cos/sin tables
    T[pos%128, blk, 0] = [cos|-sin], T[.,.,1] = [sin|cos] (64 wide, fp16).
  - device, per core (16 row tiles of 128 rows x 1024 feats):
      DMA : fp16 in/out, contiguous 2-4KB/partition transfers, all issued
            from SP (in-DMA for d+1 emitted before out-DMA of d so an
            out-DMA's sequencer wait never delays input prefetch)
      PE  : 8 fp16 matmuls [128x128x128] per row tile (y2 into PSUM fp32),
            one matmul per 2KB PSUM bank
      ACT : copy PSUM fp32 -> SBUF fp16 (2 strided copies per row tile)
      DVE : fused mul t12[t] = y * T[t] (one op, free=2048, fp16 2x mode)
      DVE/GPSIMD: crossed add out = t12_u + t12_v via strided APs (one op,
            free=1024); slow Pool add on j=0, fast DVE add last on j=1 so
            the out-DMA's final dependency resolves early
  - ramp/tail: warm-up ops preload the ACT function table and keep PE's
    p-state counter running; the first double tile is processed as two
    single row tiles; the last tile's adds run on DVE in halves with
    half-size stores.
  - Elementwise tables fold the [u|v] -> [lo|hi] pairing so a single add
    finishes the rotary: t12[0] = [u*c | -v*s], t12[1] = [u*s | v*c];
    out_lo = t12[0,u]+t12[0,v], out_hi = t12[1,u]+t12[1,v].
"""

import numpy as np

import concourse.bacc as bacc
import concourse.bass as bass
import concourse.tile as tile
from concourse import mybir
from concourse.bass_utils import run_bass_kernel_spmd

N_CORES = 8
B, S, N_STATE = 4, 4096, 1024
H, D = 16, 64           # heads, head dim
HALF = D // 2           # 32 rotary freqs
S_SH = S // N_CORES     # 512 positions per core
ROWS = B * S_SH         # 2048 rows of [1024] per core
RT = ROWS // 128        # 16 row tiles
DBL = RT // 2           # 8 double row tiles (DMA granularity)
CBLK = S_SH // 128      # 4 distinct position blocks per core
CW = 128 + CBLK * 2 * D  # combined const width (r2 | ccss)
F32 = mybir.dt.float32
F16 = mybir.dt.float16

_compiled = {}


def _build_nc():
    nc = bacc.Bacc("TRN2")
    # x pre-transposed+tiled on host: [d, p=feat%128, j=rt%2, g=chunk, r=row]
    x_in = nc.dram_tensor("x", [DBL, 128, 2, 8, 128], F16, kind="ExternalInput")
    # cst = [blockdiag(R2,R2) | ccss tables]; ccss[p, blk*128 + t*64 + f]:
    # t=0 -> [cos|-sin], t=1 -> [sin|cos] for position blk*128+p
    cst_in = nc.dram_tensor("cst", [128, CW], F16, kind="ExternalInput")
    # out[d, p=row%128, j, col]; host un-permutes
    out_d = nc.dram_tensor("out", [DBL, 128, 2, N_STATE], F16,
                           kind="ExternalOutput")

    with tile.TileContext(nc) as tc:
        with (
            tc.tile_pool(name="const", bufs=1) as const,
            tc.tile_pool(name="xin", bufs=4) as xin,
            tc.tile_pool(name="xin0", bufs=2) as xin0,
            tc.tile_pool(name="ypsum", bufs=2, space="PSUM") as ypsum,
            tc.tile_pool(name="yfp", bufs=4) as yfp,
            tc.tile_pool(name="t12p", bufs=4) as t12p,
            tc.tile_pool(name="outp", bufs=4) as outp,
        ):
            cst_sb = const.tile([128, CW], F16)
            nc.sync.dma_start(out=cst_sb[:], in_=cst_in[:])
            cst_a = cst_sb[:]
            r2_a = cst_sb[:, 0:128]

            # warm-ups: preload the ACT function table and start PE's p-state
            # clock while the first DMAs are in flight
            warm = const.tile([128, 1], F16)
            nc.vector.memset(warm[:], 0.0)
            nc.scalar.copy(out=warm[:], in_=warm[:])
            wpsum = ypsum.tile([128, 4, 512], F32, tag="yp")
            nc.tensor.matmul(wpsum[0:1, 0, 0:1], warm[:], warm[:],
                             start=True, stop=True)

            x_tiles = {}

            def prefetch(d):
                if d == 0:
                    for j in range(2):
                        x_t = xin0.tile([128, 8, 128], F16, tag="x0")
                        nc.sync.dma_start(out=x_t[:], in_=x_in[0, :, j])
                        x_tiles[(0, j)] = x_t
                else:
                    x_t = xin.tile([128, 2, 8, 128], F16, tag="x")
                    nc.sync.dma_start(out=x_t[:], in_=x_in[d])
                    x_tiles[d] = x_t

            def rowtile(xchunks, rt, og, last=False):
                """xchunks: AP-indexable [128, 8, 128]; og: out [128, 1024]."""
                c = rt % CBLK
                yf = yfp.tile([128, N_STATE], F16, tag="yf")
                for h in range(2):
                    yp = ypsum.tile([128, 4, 512], F32, tag="yp")
                    for q in range(4):
                        g = h * 4 + q
                        nc.tensor.matmul(
                            yp[:, q, 0:128], xchunks[:, g, :], r2_a,
                            start=True, stop=True,
                        )
                    # strided PSUM read packs the 4 banks' [128,128] blocks
                    ypa = yp[:]
                    yfa = yf[:]
                    nc.scalar.copy(
                        out=bass.AP(tensor=yfa.tensor,
                                    offset=yfa.offset + h * 512,
                                    ap=[list(yfa.ap[0]), [128, 4], [1, 128]]),
                        in_=bass.AP(tensor=ypa.tensor, offset=ypa.offset,
                                    ap=[list(ypa.ap[0]), [512, 4], [1, 128]]),
                    )
                # one fused DVE mul: t12[t, hd, f] = y[hd, f] * T[t, f]
                t12 = t12p.tile([128, 2, N_STATE], F16, tag="t12")
                yfa = yf[:]
                nc.vector.tensor_mul(
                    t12[:],
                    bass.AP(tensor=yfa.tensor, offset=yfa.offset,
                            ap=[list(yfa.ap[0]), [0, 2], [D, H], [1, D]]),
                    bass.AP(tensor=cst_a.tensor,
                            offset=cst_a.offset + 128 + c * 2 * D,
                            ap=[list(cst_a.ap[0]), [D, 2], [0, H], [1, D]]),
                )
                # crossed add out[t, hd, t*32:...] = t12[t, hd, u] + t12[t, hd, v]
                t12a = t12[:]

                def add(eng, f0, f1):
                    n = (f1 - f0) // D
                    eng.tensor_tensor(
                        out=bass.AP(tensor=og.tensor, offset=og.offset + f0,
                                    ap=[list(og.ap[0]), [HALF, 2], [D, n],
                                        [1, HALF]]),
                        in0=bass.AP(tensor=t12a.tensor, offset=t12a.offset + f0,
                                    ap=[list(t12a.ap[0]), [N_STATE, 2], [D, n],
                                        [1, HALF]]),
                        in1=bass.AP(tensor=t12a.tensor,
                                    offset=t12a.offset + f0 + HALF,
                                    ap=[list(t12a.ap[0]), [N_STATE, 2], [D, n],
                                        [1, HALF]]),
                        op=mybir.AluOpType.add,
                    )
                return add

            prefetch(0)
            prefetch(1)
            prefetch(2)

            # first double tile: two single row tiles for a shorter ramp
            for j in range(2):
                out_t = outp.tile([128, N_STATE], F16, tag="o0")
                add = rowtile(x_tiles.pop((0, j)), j, out_t[:])
                add(nc.gpsimd if j == 0 else nc.vector, 0, N_STATE)
                nc.sync.dma_start(out=out_d[0, :, j], in_=out_t[:])

            for d in range(1, DBL):
                x_t = x_tiles.pop(d)
                out_t = outp.tile([128, 2, N_STATE], F16, tag="o")
                last = d == DBL - 1
                for j in range(2):
                    add = rowtile(x_t[:, j], d * 2 + j, out_t[:, j, :],
                                  last=last)
                    if not last:
                        # slow Pool add first (j=0), fast DVE add last (j=1)
                        add(nc.gpsimd if j == 0 else nc.vector, 0, N_STATE)
                    elif j == 0:
                        add(nc.vector, 0, N_STATE)
                        nc.sync.dma_start(out=out_d[d, :, 0], in_=out_t[:, 0, :])
                    else:
                        # final adds in halves with half-size stores: the tail
                        # transfer after the last add is only 512 cols
                        add(nc.vector, 0, N_STATE // 2)
                        nc.sync.dma_start(out=out_d[d, :, 1, 0:512],
                                          in_=out_t[:, 1, 0:512])
                        add(nc.vector, N_STATE // 2, N_STATE)
                        nc.sync.dma_start(out=out_d[d, :, 1, 512:1024],
                                          in_=out_t[:, 1, 512:1024])
                if d + 2 < DBL:
                    prefetch(d + 2)
                if not last:
                    nc.sync.dma_start(out=out_d[d], in_=out_t[:])
    nc.compile()
    return nc


def _compose_r2(thetas, rotation_pairs, theta_scale, rotation_matrix):
    """Replicates reference._compose_rotation, then permutes cols to [even|odd]."""
    idx = rotation_pairs.astype(np.int32)
    th = (thetas.astype(np.float32) * np.float32(theta_scale[0]))
    R = np.eye(D, dtype=np.float32)
    for k in range(th.shape[0]):
        i, j = int(idx[k, 0]), int(idx[k, 1])
        ck, sk = np.float32(np.cos(th[k])), np.float32(np.sin(th[k]))
        G = np.eye(D, dtype=np.float32)
        G[i, i] = ck
        G[i, j] = -sk
        G[j, i] = sk
        G[j, j] = ck
        R = (R @ G).astype(np.float32)
    R = (R @ rotation_matrix.astype(np.float32)).astype(np.float32)
    return np.ascontiguousarray(
        np.concatenate([R[:, 0::2], R[:, 1::2]], axis=1), dtype=np.float32
    )


def make_in_maps(x, thetas, rotation_pairs, theta_scale, rotation_matrix,
                 inv_freq):
    x = np.asarray(x, dtype=np.float32)
    r2s = _compose_r2(
        np.asarray(thetas, np.float32),
        np.asarray(rotation_pairs, np.float32),
        np.asarray(theta_scale, np.float32),
        np.asarray(rotation_matrix, np.float32),
    )
    r2 = np.zeros((128, 128), dtype=np.float32)
    r2[0:D, 0:D] = r2s
    r2[D:128, D:128] = r2s

    pos = np.arange(S, dtype=np.float32)
    sinusoid = pos[:, None] * np.asarray(inv_freq, np.float32)[None, :]  # [S,32]
    cosf = np.cos(sinusoid).astype(np.float32)
    sinf = np.sin(sinusoid).astype(np.float32)

    in_maps = []
    for k in range(N_CORES):
        blk = slice(k * S_SH, (k + 1) * S_SH)
        cb, sb = cosf[blk], sinf[blk]                       # [512, 32]
        t0 = np.concatenate([cb, -sb], axis=1)              # [512, 64]
        t1 = np.concatenate([sb, cb], axis=1)
        ccss = np.stack([t0, t1], axis=1)                   # [512, 2, 64]
        ccss = ccss.reshape(CBLK, 128, 2 * D).transpose(1, 0, 2)
        cst = np.concatenate(
            [r2, ccss.reshape(128, CBLK * 2 * D)], axis=1)  # [128, CW]
        cst = np.ascontiguousarray(cst, dtype=np.float16)

        xs = x[:, blk, :].reshape(B, CBLK, 128, 8, 128)     # [b, sblk, r, g, p]
        xs = xs.transpose(0, 1, 4, 3, 2).reshape(DBL, 2, 128, 8, 128)
        xs = np.ascontiguousarray(
            xs.transpose(0, 2, 1, 3, 4), dtype=np.float16)  # [d, p, j, g, r]
        in_maps.append({"x": xs, "cst": cst})
    return in_maps


def kernel(x, thetas, rotation_pairs, theta_scale, rotation_matrix, inv_freq):
    in_maps = make_in_maps(x, thetas, rotation_pairs, theta_scale,
                           rotation_matrix, inv_freq)
    if "nc" not in _compiled:
        _compiled["nc"] = _build_nc()
    res = run_bass_kernel_spmd(_compiled["nc"], in_maps, list(range(N_CORES))).results

    out = np.empty((B, S, N_STATE), dtype=np.float32)
    for k in range(N_CORES):
        blk = slice(k * S_SH, (k + 1) * S_SH)
        o = res[k]["out"]                                   # [d, p, j, col] f16
        o = o.transpose(0, 2, 1, 3).reshape(B, S_SH, N_STATE)
        out[:, blk, :] = o.astype(np.float32)
    return out


# revision 30
# speedup vs baseline: 2.0688x; 1.0239x over previous
"""CombinedRotaryEmbedding Trainium2 kernel (fp16 I/O, host-pretransposed).

Math (per 64-dim head, position s):
    y   = x @ R2            R2 = [R_even | R_odd]  ->  y = [u(32) | v(32)]
    out = [u*cos - v*sin | u*sin + v*cos]          cos/sin = f(position, freq)

Strategy (8-way data parallel over the sequence dim, 512 positions/core):
  - host: compose R (tiny [64,64]); build blockdiag([R2,R2]) in fp16; cast +
    pre-transpose x to fp16 with features on partitions so the device needs
    no transpose pass; build per-core compact cos/sin tables
    T[pos%128, blk, 0] = [cos|-sin], T[.,.,1] = [sin|cos] (64 wide, fp16).
  - device, per core (16 row tiles of 128 rows x 1024 feats):
      DMA : fp16 in/out, contiguous 2-4KB/partition transfers, all issued
            from SP (in-DMA for d+1 emitted before out-DMA of d so an
            out-DMA's sequencer wait never delays input prefetch)
      PE  : 8 fp16 matmuls [128x128x128] per row tile (y2 into PSUM fp32),
            one matmul per 2KB PSUM bank
      ACT : copy PSUM fp32 -> SBUF fp16 (2 strided copies per row tile)
      DVE : fused mul t12[t] = y * T[t] (one op, free=2048, fp16 2x mode)
      DVE/GPSIMD: crossed add out = t12_u + t12_v via strided APs (one op,
            free=1024); slow Pool add on j=0, fast DVE add last on j=1 so
            the out-DMA's final dependency resolves early
  - ramp/tail: warm-up ops preload the ACT function table and keep PE's
    p-state counter running; the first double tile is processed as two
    single row tiles; the last tile's adds run on DVE in halves with
    half-size stores.
  - Elementwise tables fold the [u|v] -> [lo|hi] pairing so a single add
    finishes the rotary: t12[0] = [u*c | -v*s], t12[1] = [u*s | v*c];
    out_lo = t12[0,u]+t12[0,v], out_hi = t12[1,u]+t12[1,v].
"""

import numpy as np

import concourse.bacc as bacc
import concourse.bass as bass
import concourse.tile as tile
from concourse import mybir
from concourse.bass_utils import run_bass_kernel_spmd

N_CORES = 8
B, S, N_STATE = 4, 4096, 1024
H, D = 16, 64           # heads, head dim
HALF = D // 2           # 32 rotary freqs
S_SH = S // N_CORES     # 512 positions per core
ROWS = B * S_SH         # 2048 rows of [1024] per core
RT = ROWS // 128        # 16 row tiles
DBL = RT // 2           # 8 double row tiles (DMA granularity)
CBLK = S_SH // 128      # 4 distinct position blocks per core
CW = 128 + CBLK * 2 * D  # combined const width (r2 | ccss)
F32 = mybir.dt.float32
F16 = mybir.dt.float16

_compiled = {}


POOL_ADD_CUT = 4   # j=1 rowtiles below this also send their add to the Pool
SPLIT0 = True
CST_SPLIT = False


def _build_nc():
    nc = bacc.Bacc("TRN2")
    # x pre-transposed+tiled on host: [d, p=feat%128, j=rt%2, g=chunk, r=row]
    x_in = nc.dram_tensor("x", [DBL, 128, 2, 8, 128], F16, kind="ExternalInput")
    # x0r = [blockdiag(R2,R2) | x tile (0,0)]: one contiguous head DMA
    x0r_in = nc.dram_tensor("x0r", [128, 128 + 1024], F16, kind="ExternalInput")
    # cst = ccss tables; ccss[p, blk*128 + t*64 + f]:
    # t=0 -> [cos|-sin], t=1 -> [sin|cos] for position blk*128+p
    cst_in = nc.dram_tensor("cst", [128, CW - 128], F16, kind="ExternalInput")
    # out[d, p=row%128, j, col]; host un-permutes
    out_d = nc.dram_tensor("out", [DBL, 128, 2, N_STATE], F16,
                           kind="ExternalOutput")

    with tile.TileContext(nc) as tc:
        with (
            tc.tile_pool(name="const", bufs=1) as const,
            tc.tile_pool(name="xin", bufs=7) as xin,
            tc.tile_pool(name="xin0", bufs=2) as xin0,
            tc.tile_pool(name="ypsum", bufs=2, space="PSUM") as ypsum,
            tc.tile_pool(name="yfp", bufs=5) as yfp,
            tc.tile_pool(name="t12p", bufs=6) as t12p,
            tc.tile_pool(name="outp", bufs=6) as outp,
        ):
            cst_sb = const.tile([128, CW - 128], F16)
            cst_a = cst_sb[:]
            x0r = const.tile([128, 128 + 1024], F16)
            r2_a = x0r[:, 0:128]

            # warm-ups: preload the ACT function table and start PE's p-state
            # clock while the first DMAs are in flight
            warm = const.tile([128, 1], F16)
            nc.vector.memset(warm[:], 0.0)
            nc.scalar.copy(out=warm[:], in_=warm[:])
            wpsum = ypsum.tile([128, 4, 512], F32, tag="yp")
            nc.tensor.matmul(wpsum[0:1, 0, 0:1], warm[:], warm[:],
                             start=True, stop=True)

            x_tiles = {}

            def prefetch(d):
                x_t = xin.tile([128, 2, 8, 128], F16, tag="x")
                nc.sync.dma_start(out=x_t[:], in_=x_in[d])
                x_tiles[d] = x_t

            def rowtile(xchunks, rt, og, split=False):
                """xchunks: AP-indexable [128, 8, 128]; og: out [128, 1024].
                split=True runs the fused mul per half right after its copy
                (shorter ramp); otherwise one mul covers the full tile."""
                c = rt % CBLK
                yf = yfp.tile([128, N_STATE], F16, tag="yf")
                t12 = t12p.tile([128, 2, N_STATE], F16, tag="t12")
                t12w = t12[:]

                def mul(h0, h1):
                    yfa = yf[:]
                    nh = h1 - h0
                    nc.vector.tensor_mul(
                        bass.AP(tensor=t12w.tensor,
                                offset=t12w.offset + h0 * D,
                                ap=[list(t12w.ap[0]), [N_STATE, 2], [D, nh],
                                    [1, D]]),
                        bass.AP(tensor=yfa.tensor, offset=yfa.offset + h0 * D,
                                ap=[list(yfa.ap[0]), [0, 2], [D, nh], [1, D]]),
                        bass.AP(tensor=cst_a.tensor,
                                offset=cst_a.offset + c * 2 * D,
                                ap=[list(cst_a.ap[0]), [D, 2], [0, nh],
                                    [1, D]]),
                    )

                for h in range(2):
                    yp = ypsum.tile([128, 4, 512], F32, tag="yp")
                    for q in range(4):
                        g = h * 4 + q
                        nc.tensor.matmul(
                            yp[:, q, 0:128], xchunks[:, g, :], r2_a,
                            start=True, stop=True,
                        )
                    # strided PSUM read packs the 4 banks' [128,128] blocks
                    ypa = yp[:]
                    yfa = yf[:]
                    nc.scalar.copy(
                        out=bass.AP(tensor=yfa.tensor,
                                    offset=yfa.offset + h * 512,
                                    ap=[list(yfa.ap[0]), [128, 4], [1, 128]]),
                        in_=bass.AP(tensor=ypa.tensor, offset=ypa.offset,
                                    ap=[list(ypa.ap[0]), [512, 4], [1, 128]]),
                    )
                    if split:
                        mul(h * 8, h * 8 + 8)
                if not split:
                    mul(0, H)
                # crossed add out[t, hd, t*32:...] = t12[t, hd, u] + t12[t, hd, v]
                t12a = t12[:]

                def add(eng, f0, f1):
                    n = (f1 - f0) // D
                    o_ap = bass.AP(tensor=og.tensor, offset=og.offset + f0,
                                   ap=[list(og.ap[0]), [HALF, 2], [D, n],
                                       [1, HALF]])
                    u_ap = bass.AP(tensor=t12a.tensor, offset=t12a.offset + f0,
                                   ap=[list(t12a.ap[0]), [N_STATE, 2], [D, n],
                                       [1, HALF]])
                    v_ap = bass.AP(tensor=t12a.tensor,
                                   offset=t12a.offset + f0 + HALF,
                                   ap=[list(t12a.ap[0]), [N_STATE, 2], [D, n],
                                       [1, HALF]])
                    eng.tensor_tensor(out=o_ap, in0=u_ap, in1=v_ap,
                                      op=mybir.AluOpType.add)
                return add

            # head order: tiny r2 -> first input tile -> cos/sin tables ->
            # remaining inputs.  Everything is prefetched up front: input flow
            # never waits on the out-DMAs' sequencer stalls, and SBUF has room
            # for all of x.
            nc.sync.dma_start(out=x0r[:], in_=x0r_in[:])
            x_tiles[(0, 0)] = x0r[:, 128:1152].rearrange("p (g r) -> p g r", g=8)
            nc.sync.dma_start(out=cst_sb[:], in_=cst_in[:])
            x0b = xin0.tile([128, 8, 128], F16, tag="x0")
            nc.sync.dma_start(out=x0b[:], in_=x_in[0, :, 1])
            x_tiles[(0, 1)] = x0b[:]
            for d in range(1, DBL):
                prefetch(d)

            # first double tile: two single row tiles for a shorter ramp
            for j in range(2):
                out_t = outp.tile([128, N_STATE], F16, tag="o0")
                add = rowtile(x_tiles.pop((0, j)), j, out_t[:], split=SPLIT0)
                add(nc.gpsimd if j == 0 else nc.vector, 0, N_STATE)
                nc.sync.dma_start(out=out_d[0, :, j], in_=out_t[:])

            for d in range(1, DBL):
                x_t = x_tiles.pop(d)
                out_t = outp.tile([128, 2, N_STATE], F16, tag="o")
                last = d == DBL - 1
                for j in range(2):
                    add = rowtile(x_t[:, j], d * 2 + j, out_t[:, j, :])
                    if not last:
                        # Pool takes j=0 adds (plus early j=1 while DVE ramps);
                        # DVE closes each tile pair so out-DMAs resolve fast
                        pool = j == 0 or (d * 2 + j) < POOL_ADD_CUT
                        add(nc.gpsimd if pool else nc.vector, 0, N_STATE)
                    elif j == 0:
                        add(nc.gpsimd, 0, N_STATE)
                        nc.sync.dma_start(out=out_d[d, :, 0], in_=out_t[:, 0, :])
                    else:
                        # final adds in halves with half-size stores: the tail
                        # transfer after the last add is only 512 cols
                        add(nc.vector, 0, N_STATE // 2)
                        nc.sync.dma_start(out=out_d[d, :, 1, 0:512],
                                          in_=out_t[:, 1, 0:512])
                        add(nc.vector, N_STATE // 2, N_STATE)
                        nc.sync.dma_start(out=out_d[d, :, 1, 512:1024],
                                          in_=out_t[:, 1, 512:1024])
                if not last:
                    nc.sync.dma_start(out=out_d[d], in_=out_t[:])
    nc.compile()
    return nc


def _compose_r2(thetas, rotation_pairs, theta_scale, rotation_matrix):
    """Replicates reference._compose_rotation, then permutes cols to [even|odd]."""
    idx = rotation_pairs.astype(np.int32)
    th = (thetas.astype(np.float32) * np.float32(theta_scale[0]))
    R = np.eye(D, dtype=np.float32)
    for k in range(th.shape[0]):
        i, j = int(idx[k, 0]), int(idx[k, 1])
        ck, sk = np.float32(np.cos(th[k])), np.float32(np.sin(th[k]))
        G = np.eye(D, dtype=np.float32)
        G[i, i] = ck
        G[i, j] = -sk
        G[j, i] = sk
        G[j, j] = ck
        R = (R @ G).astype(np.float32)
    R = (R @ rotation_matrix.astype(np.float32)).astype(np.float32)
    return np.ascontiguousarray(
        np.concatenate([R[:, 0::2], R[:, 1::2]], axis=1), dtype=np.float32
    )


def make_in_maps(x, thetas, rotation_pairs, theta_scale, rotation_matrix,
                 inv_freq):
    x = np.asarray(x, dtype=np.float32)
    r2s = _compose_r2(
        np.asarray(thetas, np.float32),
        np.asarray(rotation_pairs, np.float32),
        np.asarray(theta_scale, np.float32),
        np.asarray(rotation_matrix, np.float32),
    )
    r2 = np.zeros((128, 128), dtype=np.float32)
    r2[0:D, 0:D] = r2s
    r2[D:128, D:128] = r2s

    pos = np.arange(S, dtype=np.float32)
    sinusoid = pos[:, None] * np.asarray(inv_freq, np.float32)[None, :]  # [S,32]
    cosf = np.cos(sinusoid).astype(np.float32)
    sinf = np.sin(sinusoid).astype(np.float32)

    in_maps = []
    for k in range(N_CORES):
        blk = slice(k * S_SH, (k + 1) * S_SH)
        cb, sb = cosf[blk], sinf[blk]                       # [512, 32]
        t0 = np.concatenate([cb, -sb], axis=1)              # [512, 64]
        t1 = np.concatenate([sb, cb], axis=1)
        ccss = np.stack([t0, t1], axis=1)                   # [512, 2, 64]
        ccss = ccss.reshape(CBLK, 128, 2 * D).transpose(1, 0, 2)
        cst = np.ascontiguousarray(
            ccss.reshape(128, CBLK * 2 * D), dtype=np.float16)

        xs = x[:, blk, :].reshape(B, CBLK, 128, 8, 128)     # [b, sblk, r, g, p]
        xs = xs.transpose(0, 1, 4, 3, 2).reshape(DBL, 2, 128, 8, 128)
        xs = np.ascontiguousarray(
            xs.transpose(0, 2, 1, 3, 4), dtype=np.float16)  # [d, p, j, g, r]
        x0r = np.concatenate(
            [r2.astype(np.float16), xs[0, :, 0].reshape(128, 1024)], axis=1)
        in_maps.append({"x": xs, "x0r": np.ascontiguousarray(x0r), "cst": cst})
    return in_maps


def kernel(x, thetas, rotation_pairs, theta_scale, rotation_matrix, inv_freq):
    in_maps = make_in_maps(x, thetas, rotation_pairs, theta_scale,
                           rotation_matrix, inv_freq)
    if "nc" not in _compiled:
        _compiled["nc"] = _build_nc()
    res = run_bass_kernel_spmd(_compiled["nc"], in_maps, list(range(N_CORES))).results

    out = np.empty((B, S, N_STATE), dtype=np.float32)
    for k in range(N_CORES):
        blk = slice(k * S_SH, (k + 1) * S_SH)
        o = res[k]["out"]                                   # [d, p, j, col] f16
        o = o.transpose(0, 2, 1, 3).reshape(B, S_SH, N_STATE)
        out[:, blk, :] = o.astype(np.float32)
    return out
